# revision 26
# baseline (speedup 1.0000x reference)
"""Trainium2 Bass kernel for nn_CMPModel (complex density matrix).

Math (per batch b, S=128 tokens, D=256):
    R = word_emb[questions[b]]                # [S, D]
    I = cmp_emb[questions[b]] * pos[b][:, None]
    real = R^T W R + I^T W I                  # symmetric   (W = diag(weighted_q))
    imag = I^T W R - R^T W I                  # antisymmetric

We compute only C = real + imag on device: two PSUM-accumulated products
with 3 prepped operand tiles per batch:
    C = A^T r + B^T wposc
      wposc = (w*pos)*c
      A     = w*r + wposc
      B     = pos*c - r
Host recovers (exact by symmetry):  real = (C + C^T)/2,  imag = (C - C^T)/2.

Sharding: data-parallel over batch, 8 batches per core.

v3 structure (perfetto-trace driven):
  - COMPACT per-core table: host dedups the <=1024 rows this core touches
    (np.unique) into [1024, 512] bf16; indices remapped to the compact id.
    Halves gather bytes (1KB rows) and shrinks upload 102MB -> 1MB/core.
  - 8 single-offset indirect gathers (one per batch). Multi-offset forms
    were HW-probed and scramble at sub-run granularity (descriptor-level
    offset pairing); single-offset is exact. SWDGE desc-gen is ~9ns/row,
    so the gather stream is ~1.2-1.4us/batch of serial Q7 time - the
    pacing resource. All consts ride ONE gpsimd DMA (blob: idx|pos|wq
    bitcast) so the first gather issues as early as possible.
  - bf16 matmuls (fp32 HIGH mode was 4x slower) + PE WARM-UP: HAM clock
    gating runs the PE at 1.2GHz until ~3.4us of sustained activity, so
    dummy matmuls issued during the gather wait bring it to 2.4GHz
    before the real MM stream starts.
  - ACT table preload via an early dummy copy (v2 showed ACT_TABLE_LOAD
    landing on the critical path otherwise).
  - bf16 prep on DVE (3 ops/batch ~1.1us < gather cadence), PSUM->SBUF
    copies with bf16 cast on ACT, per-batch bf16 out DMA on sync.
Accuracy: bf16 quantization of table + prep + C ~ 4e-3 rel (gate 2e-2).
"""

import ml_dtypes
import numpy as np

import concourse.bacc as bacc
import concourse.bass as bass
import concourse.mybir as mybir
import concourse.tile as tile
from concourse.bass_utils import run_bass_kernel_spmd

V, D, S, B = 50000, 256, 128, 64
NCORES = 8
NB = B // NCORES          # batches per core
P = 128
U = NB * P                # compacted table rows (padded)
NWARM = 12                # PE warm-up matmuls (N=512 each, ~5us cold span)
F32 = mybir.dt.float32
BF16 = mybir.dt.bfloat16
I32 = mybir.dt.int32
MUL = mybir.AluOpType.mult
ADD = mybir.AluOpType.add
SUB = mybir.AluOpType.subtract

# set by test harness: trace the run and stash exec_time_ns
TRACE = False
LAST_EXEC_NS = None
LAST_RESULTS = None


def _emit_copy_out(nc, outp, out_d, ps2, j):
    # one copy + one DMA per PAIR of batches (2-bank PSUM tile): halves
    # the per-op fixed overhead on the copy engines vs per-batch copies
    out_sb = outp.tile([P, 2, 2, D], BF16, tag="osb", name=f"osb{j}")
    if j % 2 == 0:
        nc.vector.tensor_copy(out_sb[:], ps2[j][:])
    else:
        nc.scalar.copy(out_sb[:], ps2[j][:])
    nc.sync.dma_start(out=out_d[j], in_=out_sb[:])


def build_bass():
    # no core-id branching in this kernel; dropping the partition_id input
    # removes its preamble TENSOR_LOAD if the framework ties them together
    nc = bacc.Bacc("TRN2", enable_partition_id=False)
    tables = nc.declare_dram_parameter("tables", [U, 2 * D], BF16, isOutput=False)
    # blob: cols 0..7 idx (int32), 8..15 pos (f32 bits), 16 wq (f32 bits)
    blob_d = nc.declare_dram_parameter("blob", [P, 17], I32, isOutput=False)
    # pair-major layout: outc[j, p, b2, m, :] = C_{2j+b2}[m*128+p, :]
    out_d = nc.declare_dram_parameter("outc", [NB // 2, P, 2, 2, D], BF16, isOutput=True)

    with tile.TileContext(nc) as tc:
        with (
            tc.tile_pool(name="const", bufs=1) as constp,
            tc.tile_pool(name="gather", bufs=1) as gatherp,
            tc.tile_pool(name="work", bufs=8) as workp,
            tc.tile_pool(name="outp", bufs=8) as outp,
            tc.tile_pool(name="psum", bufs=4, space="PSUM") as psump,
        ):
            blob = constp.tile([P, 17], I32)
            # blob on ACT's HWDGE: its queue reaches the DMA ~0.5us before
            # sync's does, so the first gather (which waits on the blob
            # sem) issues earlier. gpsimd's first instruction is gather 0.
            nc.scalar.dma_start(out=blob[:], in_=blob_d[:])
            pos = blob[:, 8:16].bitcast(F32)    # [P, NB] f32 view
            wq = blob[:, 16:17].bitcast(F32)    # [P, 1] f32 view

            # PE warm-up source + ACT table preload, both dependency-free
            warm = constp.tile([P, 2 * D], BF16)
            nc.vector.memset(warm[:], 0.0)
            preload = constp.tile([P, 8], BF16)
            nc.scalar.copy(preload[:], warm[:, 0:8])  # forces ACT_TABLE_LOAD early

            # w*pos per (token, batch), f32 (scalar operand for wposc)
            wpos = constp.tile([P, NB], F32)
            nc.vector.tensor_scalar_mul(wpos[:], pos, wq[:, :1])

            # all gathers up front: Q7 desc-gen is the serial pacing resource
            rc = gatherp.tile([P, NB, 2 * D], BF16)
            for b in range(NB):
                nc.gpsimd.indirect_dma_start(
                    out=rc[:, b, :],
                    out_offset=None,
                    in_=tables[:],
                    in_offset=bass.IndirectOffsetOnAxis(
                        ap=blob[:, b : b + 1], axis=0
                    ),
                )

            # PSUM: one 2-bank tile per batch PAIR (ps2[j][:, b%2, m, :] is
            # one matmul target); warm-ups hit the last pair's banks and
            # are overwritten by its start=True matmuls later.
            ps2 = []
            for _ in range(NB // 2):
                ps = psump.tile([P, 2, 2, D], F32, space="PSUM", tag="ps")
                ps2.append(ps)
            for i in range(NWARM):
                nc.tensor.matmul(
                    ps2[-1][:, 0, :, :], warm[:, 0:P], warm[:], start=True, stop=True
                )

            # per-batch pipeline behind the gather stream
            for b in range(NB):
                r_b = rc[:, b, 0:D]
                c_b = rc[:, b, D : 2 * D]
                wposc = workp.tile([P, D], BF16, tag="wposc")
                a_t = workp.tile([P, D], BF16, tag="a")
                b_t = workp.tile([P, D], BF16, tag="b")
                # wposc on ACT (per-partition scale mul) frees DVE; b_t
                # runs concurrently; a_t last (needs wposc).
                nc.scalar.mul(wposc[:], c_b, wpos[:, b : b + 1])
                nc.vector.scalar_tensor_tensor(
                    b_t[:], c_b, pos[:, b : b + 1], r_b, MUL, SUB
                )
                nc.vector.scalar_tensor_tensor(
                    a_t[:], r_b, wq[:, :1], wposc[:], MUL, ADD
                )

                ps = ps2[b // 2]
                # B-product first: its operands (b_t, wposc) are ready
                # before a_t, shortening the per-batch (and tail) chain.
                for m in range(2):
                    msl = slice(m * P, (m + 1) * P)
                    nc.tensor.matmul(
                        ps[:, b % 2, m, :], b_t[:, msl], wposc[:], start=True, stop=False
                    )
                    nc.tensor.matmul(
                        ps[:, b % 2, m, :], a_t[:, msl], r_b, start=False, stop=True
                    )

                # copy+DMA of the PREVIOUS pair emitted here (its matmuls
                # are long done, so the copy never stalls the in-order
                # DVE/ACT queues in front of later batches' prep).
                if b >= 3 and b % 2 == 1:
                    _emit_copy_out(nc, outp, out_d, ps2, b // 2 - 1)
            # last pair: per-BATCH copies+DMAs on ACT - a 0.6us copy on the
            # tail chain instead of a 1.1us pair copy, and batch 6's output
            # is already in flight while batch 7 finishes.
            jl = NB // 2 - 1
            for b2 in range(2):
                out_sb = outp.tile([P, 2, D], BF16, tag="osb", name=f"osbl{b2}")
                nc.scalar.copy(out_sb[:], ps2[jl][:, b2, :, :])
                nc.sync.dma_start(out=out_d[jl][:, b2], in_=out_sb[:])
    nc.compile()
    return nc


_NC = None


def _get_nc():
    global _NC
    if _NC is None:
        _NC = build_bass()
    return _NC


def make_in_map(questions_core, q_position_core, word_emb, cmp_emb, weighted_q):
    """One core's inputs: compacted bf16 table + const blob (idx|pos|wq)."""
    uniq, inv = np.unique(questions_core, return_inverse=True)
    inv = inv.reshape(NB, P)                  # [b, p] -> compact row id
    tbl = np.zeros((U, 2 * D), dtype=ml_dtypes.bfloat16)
    tbl[: len(uniq), :D] = word_emb[uniq]
    tbl[: len(uniq), D:] = cmp_emb[uniq]
    blob = np.empty((P, 17), dtype=np.int32)
    blob[:, 0:8] = inv.T.astype(np.int32)
    blob[:, 8:16] = q_position_core.T.astype(np.float32).view(np.int32)
    blob[:, 16] = weighted_q.astype(np.float32).view(np.int32)
    return {
        "tables": np.ascontiguousarray(tbl),
        "blob": np.ascontiguousarray(blob),
    }


def kernel(questions, q_position, word_emb, cmp_emb, weighted_q):
    global LAST_EXEC_NS, LAST_RESULTS
    questions = np.asarray(questions)
    q_position = np.asarray(q_position, dtype=np.float32)
    word_emb = np.asarray(word_emb, dtype=np.float32)
    cmp_emb = np.asarray(cmp_emb, dtype=np.float32)
    weighted_q = np.asarray(weighted_q, dtype=np.float32)

    in_maps = []
    for core in range(NCORES):
        bs = slice(core * NB, (core + 1) * NB)
        in_maps.append(
            make_in_map(
                questions[bs], q_position[bs], word_emb, cmp_emb, weighted_q
            )
        )

    nc = _get_nc()
    res = run_bass_kernel_spmd(nc, in_maps, list(range(NCORES)), trace=TRACE)
    LAST_EXEC_NS = res.exec_time_ns
    LAST_RESULTS = res

    # [NCORES, NB//2, P, 2, 2, D] -> C [B, 256, 256] with row d = m*128 + p
    outc = np.stack(
        [np.asarray(res.results[c]["outc"], dtype=np.float32) for c in range(NCORES)],
        axis=0,
    )
    c_all = (
        outc.reshape(NCORES, NB // 2, P, 2, 2, D)
        .transpose(0, 1, 3, 4, 2, 5)       # core, pair, b2, m, p, d
        .reshape(B, 2 * P, D)
    )
    ct = c_all.transpose(0, 2, 1)
    real = ((c_all + ct) * 0.5).astype(np.float32)
    imag = ((c_all - ct) * 0.5).astype(np.float32)
    return real, imag
